# revision 2
# baseline (speedup 1.0000x reference)
"""Trainium2 Bass kernel for nn_DiceLossLayer — D-histogram + TRI-prefix rewrite.

Data-parallel over batch: 8 masks per core on 8 cores. Per scanline y the
filled-mask count A(y,x) for x = 8J+L is decomposed as

  A(8J+L) = C0(J) + #{e : ch_e == J and xf_e < L}
    ch = floor(xint/8), xf = xint mod 8, C0(J) = #{e : ch < J}

The second term comes from ONE PE matmul per scanline: stationary
D[e,J] = [ch8_e == 8J] (32 is_equal blocks), moving CUM[e,L] = [xf_e < L]
(L=0 col is all-0, L=8 col all-1 -> memset once; L=1..7 compares).
The psum column L=8 then holds the per-J histogram H[J] = #{ch==J};
C0 = J-prefix of H is added by a second matmul per batch-tile with a
constant block-diagonal strict-lower-triangular stationary BD[128,128]
(start=False accumulate), with moving = H (copied to SBUF) broadcast
over the 9 L columns via a stride-0 AP.

Geometry per batch: xint in f32 (slope/beta host-precomputed), converted
to fp16; crossing exclusion via max with BIGS; xf via the exact fp16
`mod` ALU; ch8 = xs - xf.

Postproc per psum tile: u = act(ps*0.5 + 1023.75) which is exactly
1024 + floor(A/2) by round-half-even; r2 = (u-1024)*2 = 2*floor(A/2);
mask(x) = [r2(x) < A(x+1)] (equivalent to parity OR boundary since A is
monotone); T and I accumulated via stt accum_out against the q tile,
which is DMA-loaded from DRAM directly in the mask's permuted layout
and sign-thresholded on ACT (also yielding Qs via accum_out).
"""

import os

import numpy as np

os.environ.setdefault("JAX_PLATFORMS", "")

import concourse.bacc as bacc
import concourse.bass as bass
import concourse.tile as tile
from concourse import mybir
from concourse.bass_utils import run_bass_kernel_spmd

F32 = mybir.dt.float32
F16 = mybir.dt.float16
ALU = mybir.AluOpType
AF = mybir.ActivationFunctionType

N_CORES = 8
B = 8            # batches per core
NV = 128         # polygon vertices (= edges)
GRID = 256
G = 4            # batches per factor round
NJ = 32          # x-high digit J on psum partitions (x = 8J + L)
NL = 9           # A columns per slot: L = 0..8 (L=8 -> next block boundary = H)
S0 = 56          # y-slots per quadrant, first tile of a batch (224 y)
S1 = 8           # y-slots per quadrant, second tile (32 y)
YW = G * GRID
SMOOTH = 1e-6
BIGS = 10000.0   # fp16-exact sentinel for non-crossing edges


def _q_thresh() -> float:
    # largest f32 d with fl(d * 255f) <= 127f
    d = np.float32(127.0) / np.float32(255.0)
    one = np.float32(1.0)
    while np.nextafter(d, one) * np.float32(255.0) <= np.float32(127.0):
        d = np.nextafter(d, one)
    return float(d)


Q_THR_P = float(np.nextafter(np.float32(_q_thresh()), np.float32(1.0)))

_CACHE = {}


def _emit(ctx, tc, prm_d, dmap_d, stats_d):
    nc = tc.nc

    setup = ctx.enter_context(tc.tile_pool(name="setup", bufs=1))
    tmp = ctx.enter_context(tc.tile_pool(name="tmp", bufs=1))
    qp = ctx.enter_context(tc.tile_pool(name="qp", bufs=2))
    post = ctx.enter_context(tc.tile_pool(name="post", bufs=2))
    psum = ctx.enter_context(tc.tile_pool(name="psum", bufs=3, space="PSUM"))
    psfin = ctx.enter_context(tc.tile_pool(name="psfin", bufs=1, space="PSUM"))

    # ---------------- setup: params + iotas + constants ----------------
    sb_prm = setup.tile([NV, 4 * B], F32)
    nc.sync.dma_start(sb_prm[:], prm_d[:])
    prm4 = sb_prm.rearrange("p (b k) -> p b k", k=4)

    ioty32 = setup.tile([128, GRID], F32)
    nc.gpsimd.iota(ioty32[:], pattern=[[1, GRID]], base=0, channel_multiplier=0,
                   allow_small_or_imprecise_dtypes=True)
    ioty16 = setup.tile([128, GRID], F16)
    nc.vector.tensor_copy(ioty16[:], ioty32[:])

    # stats: T(b,t), I2(b,t), Qs(b,t) -> 3 * 16 columns
    NSTAT = 48
    sb_stats = setup.tile([128, NSTAT], F32)
    nc.vector.memset(sb_stats[:], 0.0)
    sb_onescol = setup.tile([128, 1], F32)
    nc.vector.memset(sb_onescol[:], 1.0)
    sb_qthr = setup.tile([128, 1], F32)
    nc.vector.memset(sb_qthr[:], Q_THR_P)

    # block-diagonal strict-lower-triangular BD[p, c] =
    #   [c//32 == p//32] * [c%32 > p%32]   (fp16 0/1)
    io128 = setup.tile([128, 128], F32)
    nc.gpsimd.iota(io128[:], pattern=[[1, 128]], base=0, channel_multiplier=0,
                   allow_small_or_imprecise_dtypes=True)
    pcol = setup.tile([128, 1], F32)
    nc.gpsimd.iota(pcol[:], pattern=[[0, 1]], base=0, channel_multiplier=1,
                   allow_small_or_imprecise_dtypes=True)
    def emit_div32(dst_g, dst_mod, srct, w):
        # dst_g = 32*floor(src/32), dst_mod = src - dst_g (f32 round trick)
        m = setup.tile([128, w], F32)
        nc.vector.tensor_scalar(m[:], srct[:], 1.0 / 32.0, 8388608.0,
                                ALU.mult, ALU.add)
        r = setup.tile([128, w], F32)
        nc.vector.tensor_scalar(r[:], m[:], -8388608.0, None, ALU.add)
        m2 = setup.tile([128, w], F32)
        nc.vector.tensor_scalar(m2[:], srct[:], 1.0 / 32.0, None, ALU.mult)
        d = setup.tile([128, w], F32)
        nc.vector.tensor_tensor(d[:], r[:], m2[:], ALU.is_gt)
        fl = setup.tile([128, w], F32)
        nc.vector.tensor_tensor(fl[:], r[:], d[:], ALU.subtract)
        nc.vector.tensor_scalar(dst_g[:], fl[:], 32.0, None, ALU.mult)
        nc.vector.scalar_tensor_tensor(dst_mod[:], dst_g[:], -1.0, srct[:],
                                       ALU.mult, ALU.add)

    jmod = setup.tile([128, 128], F32)
    jg = setup.tile([128, 128], F32)
    emit_div32(jg, jmod, io128, 128)
    pmod = setup.tile([128, 1], F32)
    pg = setup.tile([128, 1], F32)
    emit_div32(pg, pmod, pcol, 1)
    t1 = setup.tile([128, 128], F16)
    nc.vector.tensor_scalar(t1[:], jg[:], pg[:, 0:1], None, ALU.is_equal)
    t2 = setup.tile([128, 128], F16)
    nc.vector.tensor_scalar(t2[:], jmod[:], pmod[:, 0:1], None, ALU.is_gt)
    sb_BD = setup.tile([128, 128], F16)
    nc.vector.tensor_tensor(sb_BD[:], t1[:], t2[:], ALU.mult)

    # factor tiles, double-buffered across rounds; L=0/L=8 CUM blocks constant
    sb_D2 = [setup.tile([128, NJ * YW], F16, name=f"D{r}") for r in range(2)]
    sb_CUM2 = [setup.tile([128, NL * YW], F16, name=f"CUM{r}") for r in range(2)]
    for r in range(2):
        nc.vector.memset(sb_CUM2[r][:, 0:YW], 0.0)
        nc.gpsimd.memset(sb_CUM2[r][:, 8 * YW : 9 * YW], 1.0)

    # geometry round tiles (double-buffered)
    sb_xs2 = [setup.tile([128, YW], F16, name=f"xs{r}") for r in range(2)]
    sb_xf2 = [setup.tile([128, YW], F16, name=f"xf{r}") for r in range(2)]
    sb_ch82 = [setup.tile([128, YW], F16, name=f"ch8{r}") for r in range(2)]

    dmv = dmap_d[:]

    def emit_qpath(b):
        # q in the mask's permuted layout, straight from DRAM (f32):
        # qp0[32g+J, 8i+L] = dmap[b, 56g+i, 8J+L]; qp1 for y >= 224.
        qp0 = qp.tile([128, S0 * 8], F32, tag="qp0")
        qp1 = qp.tile([32, 32 * 8], F32, tag="qp1")
        import os
        if os.environ.get("KCUT") == "noqdma":
            nc.vector.memset(qp0[:], 0.5)
            nc.vector.memset(qp1[:], 0.5)
        ps_p = qp0.ap[0][0]
        for g in range([0, 4][os.environ.get("KCUT") != "noqdma"]):
            src0 = bass.AP(tensor=dmv.tensor,
                           offset=dmv.offset + b * 65536 + g * S0 * GRID,
                           ap=[[8, NJ], [GRID, S0], [1, 8]])
            dst0 = bass.AP(tensor=qp0.tensor, offset=qp0.offset + 32 * g * ps_p,
                           ap=[[ps_p, NJ], [8, S0], [1, 8]])
            eng = nc.sync if (b + g) % 2 == 0 else nc.scalar
            eng.dma_start(dst0, src0)
        if os.environ.get("KCUT") != "noqdma":
            src1 = bass.AP(tensor=dmv.tensor,
                           offset=dmv.offset + b * 65536 + 4 * S0 * GRID,
                           ap=[[8, NJ], [GRID, 32], [1, 8]])
            eng2 = nc.scalar if b % 2 == 0 else nc.sync
            eng2.dma_start(qp1[:], src1)
        qs0 = qp.tile([128, S0 * 8], F16, tag="qs0")
        nc.scalar.activation(qs0[:], qp0[:], AF.Sign, bias=sb_qthr[:, 0:1],
                             scale=-1.0,
                             accum_out=sb_stats[:, 32 + 2 * b : 33 + 2 * b])
        qs1 = qp.tile([32, 32 * 8], F16, tag="qs1")
        nc.scalar.activation(qs1[:], qp1[:], AF.Sign, bias=sb_qthr[0:32, 0:1],
                             scale=-1.0,
                             accum_out=sb_stats[0:32, 33 + 2 * b : 34 + 2 * b])
        return qs0, qs1

    def y_to_gs(y):
        if y < 4 * S0:
            return 0, y // S0, y % S0
        return 1, 0, y - 4 * S0

    qs_tiles = {}

    # ---------------- main: software-pipelined rounds ----------------
    def emit_build(rnd):
        sb_D = sb_D2[rnd % 2]
        sb_CUM = sb_CUM2[rnd % 2]
        sb_xs = sb_xs2[rnd % 2]
        sb_xf = sb_xf2[rnd % 2]
        sb_ch8 = sb_ch82[rnd % 2]
        for bb in range(G):
            b = rnd * G + bb
            qs_tiles[b] = emit_qpath(b)
        for bb in range(G):
            b = rnd * G + bb
            sl = slice(bb * GRID, (bb + 1) * GRID)
            miny = prm4[:, b, 0:1]
            maxy = prm4[:, b, 1:2]
            slope = prm4[:, b, 2:3]
            beta = prm4[:, b, 3:4]
            xint32 = tmp.tile([128, GRID], F32, tag="xint32")
            nc.gpsimd.tensor_scalar(xint32[:], ioty32[:], slope, beta,
                                    ALU.mult, ALU.add)
            xint16 = tmp.tile([128, GRID], F16, tag="xint16")
            nc.gpsimd.tensor_scalar(xint16[:], xint32[:], 300.0, -300.0,
                                    ALU.min, ALU.max)
            n1 = tmp.tile([128, GRID], F16, tag="n1")
            nc.gpsimd.tensor_scalar(n1[:], ioty16[:], miny, BIGS, ALU.is_le,
                                    ALU.mult)
            n2 = tmp.tile([128, GRID], F16, tag="n2")
            nc.gpsimd.tensor_scalar(n2[:], ioty16[:], maxy, BIGS, ALU.is_gt,
                                    ALU.mult)
            nx = tmp.tile([128, GRID], F16, tag="nx")
            nc.vector.tensor_tensor(nx[:], n1[:], n2[:], ALU.max)
            nc.vector.tensor_tensor(sb_xs[:, sl], xint16[:], nx[:], ALU.max)
            # ch8 = 8*floor(xs/8), xf = xs - ch8, exactly in fp16:
            # u2 = round_half_even(xs/8) + 2048 ; correct round->floor via d2
            u2 = tmp.tile([128, GRID], F16, tag="u2")
            nc.gpsimd.tensor_scalar(u2[:], sb_xs[:, sl], 0.125, 2048.0,
                                    ALU.mult, ALU.add)
            ch8r = tmp.tile([128, GRID], F16, tag="ch8r")
            nc.gpsimd.tensor_scalar(ch8r[:], u2[:], -2048.0, 8.0, ALU.add,
                                    ALU.mult)
            d2 = tmp.tile([128, GRID], F16, tag="d2")
            nc.vector.tensor_tensor(d2[:], ch8r[:], sb_xs[:, sl], ALU.is_gt)
            nc.vector.scalar_tensor_tensor(sb_ch8[:, sl], d2[:], -8.0,
                                           ch8r[:], ALU.mult, ALU.add)
            nc.gpsimd.tensor_tensor(sb_xf[:, sl], sb_xs[:, sl],
                                    sb_ch8[:, sl], ALU.subtract)
        for j in range(NJ):
            nc.vector.tensor_scalar(sb_D[:, j * YW : (j + 1) * YW], sb_ch8[:],
                                    float(8 * j), None, ALU.is_equal)
        for L in range(1, 8):
            nc.gpsimd.tensor_scalar(sb_CUM[:, L * YW : (L + 1) * YW],
                                    sb_xf[:], float(L), None, ALU.is_lt)

    def emit_mms(rnd):
        sb_D = sb_D2[rnd % 2]
        sb_CUM = sb_CUM2[rnd % 2]
        dap = sb_D[:]
        cap = sb_CUM[:]
        out = []
        for bb in range(G):
            b = rnd * G + bb
            ps0 = psum.tile([128, 512], F32, tag="A0", name=f"A0_{b}")
            ps1 = psum.tile([128, 512], F32, tag="A1", name=f"A1_{b}")
            ps_tiles = (ps0, ps1)
            started = [[False] * 4, [False] * 4]
            for y in range(GRID):
                th, g, s = y_to_gs(y)
                ps = ps_tiles[th]
                off = bb * GRID + y
                stat = bass.AP(tensor=dap.tensor, offset=dap.offset + off,
                               ap=[list(dap.ap[0]), [YW, NJ]])
                mov = bass.AP(tensor=cap.tensor, offset=cap.offset + off,
                              ap=[list(cap.ap[0]), [YW, NL]])
                nc.tensor.matmul(ps[32 * g : 32 * g + 32, NL * s : NL * s + NL],
                                 stat, mov, start=not started[th][g],
                                 stop=False, tile_position=(0, 32 * g),
                                 skip_group_check=True)
                started[th][g] = True
            # H extraction + TRI prefix accumulate (keeps PE/ACT latency low)
            tri = []
            for th, npart, ns in ((1, 32, 32), (0, 128, S0)):
                ps = ps_tiles[th]
                W_ = NL * ns
                hsb = post.tile([npart, ns], F16, tag=f"h_{th}")
                hsrc = bass.AP(tensor=ps.tensor, offset=ps.offset + 8,
                               ap=[[ps.ap[0][0], npart], [NL, ns]])
                nc.scalar.activation(hsb[:], hsrc, AF.Copy, bias=0.0,
                                     scale=1.0)
                movh = bass.AP(tensor=hsb.tensor, offset=hsb.offset,
                               ap=[[hsb.ap[0][0], npart], [1, ns], [0, NL]])
                nc.tensor.matmul(ps[0:npart, 0:W_], sb_BD[0:npart, 0:npart],
                                 movh, start=False, stop=True,
                                 tile_position=(0, 0), skip_group_check=True)
            out.append(ps_tiles)
            emit_post_batch(b, ps_tiles)
        return out

    def emit_post_batch(b, ps_tiles):
            qs_pair = qs_tiles.pop(b)
            for th, npart, ns in ((1, 32, 32), (0, 128, S0)):
                ps = ps_tiles[th]
                W_ = NL * ns
                A16 = post.tile([npart, W_], F16, tag=f"A16_{th}")
                nc.scalar.activation(A16[:], ps[0:npart, 0:W_], AF.Copy,
                                     bias=0.0, scale=1.0)
                r2 = post.tile([npart, W_], F16, tag=f"r2_{th}")
                nc.vector.tensor_scalar(r2[:], A16[:], 0.5, 1023.75, ALU.mult,
                                        ALU.add)
                nc.vector.tensor_scalar(r2[:], r2[:], -1024.0, 2.0, ALU.add,
                                        ALU.mult)
                r3 = r2.rearrange("p (s l) -> p s l", l=NL)
                a3 = A16.rearrange("p (s l) -> p s l", l=NL)
                mask = post.tile([npart, ns * 8], F16, tag=f"mask_{th}")
                t = 2 * b + th
                nc.vector.scalar_tensor_tensor(
                    mask.rearrange("p (s l) -> p s l", l=8),
                    r3[:, :, 0:8], 0.0, a3[:, :, 1:9], ALU.add, ALU.is_lt,
                    accum_out=sb_stats[0:npart, t : t + 1])
                nc.vector.scalar_tensor_tensor(
                    r2[0:npart, 0 : ns * 8], mask[:], 0.0, qs_pair[th][:],
                    ALU.add, ALU.mult,
                    accum_out=sb_stats[0:npart, 16 + t : 17 + t])

    emit_build(0)
    emit_mms(0)
    emit_build(1)
    emit_mms(1)

    # ---------------- final reduction over partitions ----------------
    ps_fin = psfin.tile([NSTAT, 1], F32, tag="fin")
    nc.tensor.matmul(ps_fin[:], sb_stats[:], sb_onescol[:], start=True,
                     stop=True)
    sb_fin = setup.tile([NSTAT, 1], F32)
    nc.vector.tensor_copy(sb_fin[:], ps_fin[:])
    nc.sync.dma_start(stats_d[:], sb_fin[:])


def _build():
    if "nc" in _CACHE:
        return _CACHE["nc"]
    nc = bacc.Bacc(None, target_bir_lowering=False, debug=False)
    prm_d = nc.dram_tensor("prm", [NV, 4 * B], F32, kind="ExternalInput")
    dmap_d = nc.dram_tensor("dmap", [B, GRID, GRID], F32, kind="ExternalInput")
    stats_d = nc.dram_tensor("stats", [48, 1], F32, kind="ExternalOutput")
    from contextlib import ExitStack

    with tile.TileContext(nc) as tc:
        with ExitStack() as ctx:
            _emit(ctx, tc, prm_d, dmap_d, stats_d)
    if hasattr(nc, "compile"):
        nc.compile()
    else:
        nc.finalize()
    _CACHE["nc"] = nc
    return nc


def _host_combine(stats: np.ndarray) -> np.ndarray:
    """stats: [48] -> 8 dice losses for this core's batches."""
    T = stats[0:16]
    I2 = stats[16:32]
    Qs = stats[32:48]
    dices = []
    for b in range(B):
        Tb = T[2 * b] + T[2 * b + 1]
        I2b = I2[2 * b] + I2[2 * b + 1]
        Ib = 0.5 * (Tb + I2b)
        Qb = 0.5 * (Qs[2 * b] + Qs[2 * b + 1] + 65536.0)
        dices.append((2.0 * Ib + SMOOTH) / (Tb + Qb + SMOOTH))
    return np.array(dices, dtype=np.float32)


def _host_params(pts: np.ndarray) -> np.ndarray:
    """pts [B, NV, 2] -> prm [NV, 4B] f32: (miny, maxy, slope, beta)."""
    pc = np.clip(pts * np.float32(255.0), np.float32(0.0),
                 np.float32(255.0)).astype(np.float32)
    pj = np.roll(pc, 1, axis=1)
    piy, pjy = pc[:, :, 1], pj[:, :, 1]
    pix, pjx = pc[:, :, 0], pj[:, :, 0]
    d = (pjy - piy).astype(np.float32)
    d = (d + (d == 0)).astype(np.float32)
    slope = np.clip((pjx - pix) / d, -1e20, 1e20).astype(np.float32)
    beta = (pix - piy * slope).astype(np.float32)
    miny = np.minimum(piy, pjy)
    maxy = np.maximum(piy, pjy)
    prm = np.stack([miny, maxy, slope, beta], axis=2)  # [B, NV, 4]
    return np.ascontiguousarray(prm.transpose(1, 0, 2).reshape(NV, 4 * B))


def kernel(points: np.ndarray, dmap: np.ndarray) -> np.ndarray:
    pts = np.asarray(points, dtype=np.float32).reshape(64, NV, 2)
    dm = np.asarray(dmap, dtype=np.float32).reshape(64, GRID, GRID)

    in_maps = []
    for r in range(N_CORES):
        sl = slice(r * B, (r + 1) * B)
        in_maps.append({
            "prm": _host_params(pts[sl]),
            "dmap": np.ascontiguousarray(dm[sl]),
        })

    nc = _build()
    res = run_bass_kernel_spmd(nc, in_maps, core_ids=list(range(N_CORES)))

    dices = []
    for r in range(N_CORES):
        s = np.asarray(res.results[r]["stats"], dtype=np.float32).reshape(-1)
        dices.append(_host_combine(s))
    dices = np.concatenate(dices).astype(np.float32)
    return np.float32(np.mean(np.float32(1.0) - dices))


# revision 3
# speedup vs baseline: 1.0159x; 1.0159x over previous
"""Trainium2 Bass kernel for nn_DiceLossLayer — D-histogram + TRI-prefix rewrite.

Data-parallel over batch: 8 masks per core on 8 cores. Per scanline y the
filled-mask count A(y,x) for x = 8J+L is decomposed as

  A(8J+L) = C0(J) + #{e : ch_e == J and xf_e < L}
    ch = floor(xint/8), xf = xint mod 8, C0(J) = #{e : ch < J}

The second term comes from ONE PE matmul per scanline: stationary
D[e,J] = [ch8_e == 8J] (32 is_equal blocks), moving CUM[e,L] = [xf_e < L]
(L=0 col is all-0, L=8 col all-1 -> memset once; L=1..7 compares).
The psum column L=8 then holds the per-J histogram H[J] = #{ch==J};
C0 = J-prefix of H is added by a second matmul per batch-tile with a
constant block-diagonal strict-lower-triangular stationary BD[128,128]
(start=False accumulate), with moving = H (copied to SBUF) broadcast
over the 9 L columns via a stride-0 AP.

Geometry per batch: xint in f32 (slope/beta host-precomputed), converted
to fp16; crossing exclusion via max with BIGS; xf via the exact fp16
`mod` ALU; ch8 = xs - xf.

Postproc per psum tile: u = act(ps*0.5 + 1023.75) which is exactly
1024 + floor(A/2) by round-half-even; r2 = (u-1024)*2 = 2*floor(A/2);
mask(x) = [r2(x) < A(x+1)] (equivalent to parity OR boundary since A is
monotone); T and I accumulated via stt accum_out against the q tile,
which is DMA-loaded from DRAM directly in the mask's permuted layout
and sign-thresholded on ACT (also yielding Qs via accum_out).
"""

import os

import numpy as np

os.environ.setdefault("JAX_PLATFORMS", "")

import concourse.bacc as bacc
import concourse.bass as bass
import concourse.tile as tile
from concourse import mybir
from concourse.bass_utils import run_bass_kernel_spmd

F32 = mybir.dt.float32
F16 = mybir.dt.float16
ALU = mybir.AluOpType
AF = mybir.ActivationFunctionType

N_CORES = 8
B = 8            # batches per core
NV = 128         # polygon vertices (= edges)
GRID = 256
G = 4            # batches per factor round
NJ = 32          # x-high digit J on psum partitions (x = 8J + L)
NL = 9           # A columns per slot: L = 0..8 (L=8 -> next block boundary = H)
S0 = 56          # y-slots per quadrant, first tile of a batch (224 y)
S1 = 8           # y-slots per quadrant, second tile (32 y)
YW = G * GRID
SMOOTH = 1e-6
BIGS = 10000.0   # fp16-exact sentinel for non-crossing edges


def _q_thresh() -> float:
    # largest f32 d with fl(d * 255f) <= 127f
    d = np.float32(127.0) / np.float32(255.0)
    one = np.float32(1.0)
    while np.nextafter(d, one) * np.float32(255.0) <= np.float32(127.0):
        d = np.nextafter(d, one)
    return float(d)


Q_THR_P = float(np.nextafter(np.float32(_q_thresh()), np.float32(1.0)))

_CACHE = {}


def _emit(ctx, tc, prm_d, dmap_d, stats_d):
    nc = tc.nc

    setup = ctx.enter_context(tc.tile_pool(name="setup", bufs=1))
    tmp = ctx.enter_context(tc.tile_pool(name="tmp", bufs=1))
    qp = ctx.enter_context(tc.tile_pool(name="qp", bufs=2))
    post = ctx.enter_context(tc.tile_pool(name="post", bufs=2))
    psum = ctx.enter_context(tc.tile_pool(name="psum", bufs=3, space="PSUM"))
    psfin = ctx.enter_context(tc.tile_pool(name="psfin", bufs=1, space="PSUM"))

    # ---------------- setup: params + iotas + constants ----------------
    sb_prm = setup.tile([NV, 4 * B], F32)
    nc.sync.dma_start(sb_prm[:], prm_d[:])
    prm4 = sb_prm.rearrange("p (b k) -> p b k", k=4)

    ioty32 = setup.tile([128, GRID], F32)
    nc.gpsimd.iota(ioty32[:], pattern=[[1, GRID]], base=0, channel_multiplier=0,
                   allow_small_or_imprecise_dtypes=True)
    ioty16 = setup.tile([128, GRID], F16)
    nc.vector.tensor_copy(ioty16[:], ioty32[:])

    # stats: T(b,t), I2(b,t), Qs(b,t) -> 3 * 16 columns
    NSTAT = 48
    sb_stats = setup.tile([128, NSTAT], F32)
    nc.vector.memset(sb_stats[:], 0.0)
    sb_onescol = setup.tile([128, 1], F32)
    nc.vector.memset(sb_onescol[:], 1.0)
    sb_qthr = setup.tile([128, 1], F32)
    nc.vector.memset(sb_qthr[:], Q_THR_P)

    # block-diagonal strict-lower-triangular BD[p, c] =
    #   [c//32 == p//32] * [c%32 > p%32]   (fp16 0/1)
    io128 = setup.tile([128, 128], F32)
    nc.gpsimd.iota(io128[:], pattern=[[1, 128]], base=0, channel_multiplier=0,
                   allow_small_or_imprecise_dtypes=True)
    pcol = setup.tile([128, 1], F32)
    nc.gpsimd.iota(pcol[:], pattern=[[0, 1]], base=0, channel_multiplier=1,
                   allow_small_or_imprecise_dtypes=True)
    def emit_div32(dst_g, dst_mod, srct, w):
        # dst_g = 32*floor(src/32), dst_mod = src - dst_g (f32 round trick)
        m = setup.tile([128, w], F32)
        nc.vector.tensor_scalar(m[:], srct[:], 1.0 / 32.0, 8388608.0,
                                ALU.mult, ALU.add)
        r = setup.tile([128, w], F32)
        nc.vector.tensor_scalar(r[:], m[:], -8388608.0, None, ALU.add)
        m2 = setup.tile([128, w], F32)
        nc.vector.tensor_scalar(m2[:], srct[:], 1.0 / 32.0, None, ALU.mult)
        d = setup.tile([128, w], F32)
        nc.vector.tensor_tensor(d[:], r[:], m2[:], ALU.is_gt)
        fl = setup.tile([128, w], F32)
        nc.vector.tensor_tensor(fl[:], r[:], d[:], ALU.subtract)
        nc.vector.tensor_scalar(dst_g[:], fl[:], 32.0, None, ALU.mult)
        nc.vector.scalar_tensor_tensor(dst_mod[:], dst_g[:], -1.0, srct[:],
                                       ALU.mult, ALU.add)

    jmod = setup.tile([128, 128], F32)
    jg = setup.tile([128, 128], F32)
    emit_div32(jg, jmod, io128, 128)
    pmod = setup.tile([128, 1], F32)
    pg = setup.tile([128, 1], F32)
    emit_div32(pg, pmod, pcol, 1)
    t1 = setup.tile([128, 128], F16)
    nc.vector.tensor_scalar(t1[:], jg[:], pg[:, 0:1], None, ALU.is_equal)
    t2 = setup.tile([128, 128], F16)
    nc.vector.tensor_scalar(t2[:], jmod[:], pmod[:, 0:1], None, ALU.is_gt)
    sb_BD = setup.tile([128, 128], F16)
    nc.vector.tensor_tensor(sb_BD[:], t1[:], t2[:], ALU.mult)

    # factor tiles, double-buffered across rounds; L=0/L=8 CUM blocks constant
    sb_D2 = [setup.tile([128, NJ * YW], F16, name=f"D{r}") for r in range(2)]
    sb_CUM2 = [setup.tile([128, NL * YW], F16, name=f"CUM{r}") for r in range(2)]
    for r in range(2):
        nc.vector.memset(sb_CUM2[r][:, 0:YW], 0.0)
        nc.gpsimd.memset(sb_CUM2[r][:, 8 * YW : 9 * YW], 1.0)

    # geometry round tiles (double-buffered)
    sb_xs2 = [setup.tile([128, YW], F16, name=f"xs{r}") for r in range(2)]
    sb_xf2 = [setup.tile([128, YW], F16, name=f"xf{r}") for r in range(2)]
    sb_ch82 = [setup.tile([128, YW], F16, name=f"ch8{r}") for r in range(2)]

    dmv = dmap_d[:]

    def emit_qpath(b):
        # q in the mask's permuted layout, straight from DRAM (f32):
        # qp0[32g+J, 8i+L] = dmap[b, 56g+i, 8J+L]; qp1 for y >= 224.
        qp0 = qp.tile([128, S0 * 8], F32, tag="qp0")
        qp1 = qp.tile([32, 32 * 8], F32, tag="qp1")
        ps_p = qp0.ap[0][0]
        for g in range(4):
            src0 = bass.AP(tensor=dmv.tensor,
                           offset=dmv.offset + b * 65536 + g * S0 * GRID,
                           ap=[[8, NJ], [GRID, S0], [1, 8]])
            dst0 = bass.AP(tensor=qp0.tensor, offset=qp0.offset + 32 * g * ps_p,
                           ap=[[ps_p, NJ], [8, S0], [1, 8]])
            eng = nc.sync if (b + g) % 2 == 0 else nc.scalar
            eng.dma_start(dst0, src0)
        src1 = bass.AP(tensor=dmv.tensor,
                       offset=dmv.offset + b * 65536 + 4 * S0 * GRID,
                       ap=[[8, NJ], [GRID, 32], [1, 8]])
        eng2 = nc.scalar if b % 2 == 0 else nc.sync
        eng2.dma_start(qp1[:], src1)
        qs0 = qp.tile([128, S0 * 8], F16, tag="qs0")
        nc.scalar.activation(qs0[:], qp0[:], AF.Sign, bias=sb_qthr[:, 0:1],
                             scale=-1.0,
                             accum_out=sb_stats[:, 32 + 2 * b : 33 + 2 * b])
        qs1 = qp.tile([32, 32 * 8], F16, tag="qs1")
        nc.scalar.activation(qs1[:], qp1[:], AF.Sign, bias=sb_qthr[0:32, 0:1],
                             scale=-1.0,
                             accum_out=sb_stats[0:32, 33 + 2 * b : 34 + 2 * b])
        return qs0, qs1

    def y_to_gs(y):
        if y < 4 * S0:
            return 0, y // S0, y % S0
        return 1, 0, y - 4 * S0

    qs_tiles = {}

    # ---------------- main: software-pipelined rounds ----------------
    def emit_build(rnd):
        sb_D = sb_D2[rnd % 2]
        sb_CUM = sb_CUM2[rnd % 2]
        sb_xs = sb_xs2[rnd % 2]
        sb_xf = sb_xf2[rnd % 2]
        sb_ch8 = sb_ch82[rnd % 2]
        for bb in range(G):
            b = rnd * G + bb
            qs_tiles[b] = emit_qpath(b)
        for bb in range(G):
            b = rnd * G + bb
            sl = slice(bb * GRID, (bb + 1) * GRID)
            miny = prm4[:, b, 0:1]
            maxy = prm4[:, b, 1:2]
            slope = prm4[:, b, 2:3]
            beta = prm4[:, b, 3:4]
            xint32 = tmp.tile([128, GRID], F32, tag="xint32")
            nc.gpsimd.tensor_scalar(xint32[:], ioty32[:], slope, beta,
                                    ALU.mult, ALU.add)
            xint16 = tmp.tile([128, GRID], F16, tag="xint16")
            nc.gpsimd.tensor_scalar(xint16[:], xint32[:], 300.0, -300.0,
                                    ALU.min, ALU.max)
            n1 = tmp.tile([128, GRID], F16, tag="n1")
            nc.gpsimd.tensor_scalar(n1[:], ioty16[:], miny, BIGS, ALU.is_le,
                                    ALU.mult)
            n2 = tmp.tile([128, GRID], F16, tag="n2")
            nc.gpsimd.tensor_scalar(n2[:], ioty16[:], maxy, BIGS, ALU.is_gt,
                                    ALU.mult)
            nx = tmp.tile([128, GRID], F16, tag="nx")
            nc.vector.tensor_tensor(nx[:], n1[:], n2[:], ALU.max)
            nc.vector.tensor_tensor(sb_xs[:, sl], xint16[:], nx[:], ALU.max)
            # ch8 = 8*floor(xs/8), xf = xs - ch8, exactly in fp16:
            # u2 = round_half_even(xs/8) + 2048 ; correct round->floor via d2
            u2 = tmp.tile([128, GRID], F16, tag="u2")
            nc.gpsimd.tensor_scalar(u2[:], sb_xs[:, sl], 0.125, 2048.0,
                                    ALU.mult, ALU.add)
            ch8r = tmp.tile([128, GRID], F16, tag="ch8r")
            nc.gpsimd.tensor_scalar(ch8r[:], u2[:], -2048.0, 8.0, ALU.add,
                                    ALU.mult)
            d2 = tmp.tile([128, GRID], F16, tag="d2")
            nc.vector.tensor_tensor(d2[:], ch8r[:], sb_xs[:, sl], ALU.is_gt)
            nc.vector.scalar_tensor_tensor(sb_ch8[:, sl], d2[:], -8.0,
                                           ch8r[:], ALU.mult, ALU.add)
            nc.gpsimd.tensor_tensor(sb_xf[:, sl], sb_xs[:, sl],
                                    sb_ch8[:, sl], ALU.subtract)
        for j in range(NJ):
            nc.vector.tensor_scalar(sb_D[:, j * YW : (j + 1) * YW], sb_ch8[:],
                                    float(8 * j), None, ALU.is_equal)
        for L in range(1, 8):
            nc.gpsimd.tensor_scalar(sb_CUM[:, L * YW : (L + 1) * YW],
                                    sb_xf[:], float(L), None, ALU.is_lt)

    def emit_mms(rnd):
        sb_D = sb_D2[rnd % 2]
        sb_CUM = sb_CUM2[rnd % 2]
        dap = sb_D[:]
        cap = sb_CUM[:]
        out = []
        for bb in range(G):
            b = rnd * G + bb
            ps0 = psum.tile([128, 512], F32, tag="A0", name=f"A0_{b}")
            ps1 = psum.tile([128, 512], F32, tag="A1", name=f"A1_{b}")
            ps_tiles = (ps0, ps1)
            started = [[False] * 4, [False] * 4]
            for y in range(GRID):
                th, g, s = y_to_gs(y)
                ps = ps_tiles[th]
                off = bb * GRID + y
                stat = bass.AP(tensor=dap.tensor, offset=dap.offset + off,
                               ap=[list(dap.ap[0]), [YW, NJ]])
                mov = bass.AP(tensor=cap.tensor, offset=cap.offset + off,
                              ap=[list(cap.ap[0]), [YW, NL]])
                nc.tensor.matmul(ps[32 * g : 32 * g + 32, NL * s : NL * s + NL],
                                 stat, mov, start=not started[th][g],
                                 stop=False, tile_position=(0, 32 * g),
                                 skip_group_check=True)
                started[th][g] = True
            # H extraction + TRI prefix accumulate (keeps PE/ACT latency low)
            tri = []
            for th, npart, ns in ((1, 32, 32), (0, 128, S0)):
                ps = ps_tiles[th]
                W_ = NL * ns
                hsb = post.tile([npart, ns], F16, tag=f"h_{th}")
                hsrc = bass.AP(tensor=ps.tensor, offset=ps.offset + 8,
                               ap=[[ps.ap[0][0], npart], [NL, ns]])
                nc.scalar.activation(hsb[:], hsrc, AF.Copy, bias=0.0,
                                     scale=1.0)
                movh = bass.AP(tensor=hsb.tensor, offset=hsb.offset,
                               ap=[[hsb.ap[0][0], npart], [1, ns], [0, NL]])
                nc.tensor.matmul(ps[0:npart, 0:W_], sb_BD[0:npart, 0:npart],
                                 movh, start=False, stop=True,
                                 tile_position=(0, 0), skip_group_check=True)
            out.append(ps_tiles)
            emit_post_batch(b, ps_tiles)
        return out

    def emit_post_batch(b, ps_tiles):
            qs_pair = qs_tiles.pop(b)
            for th, npart, ns in ((1, 32, 32), (0, 128, S0)):
                ps = ps_tiles[th]
                W_ = NL * ns
                A16 = post.tile([npart, W_], F16, tag=f"A16_{th}")
                nc.scalar.activation(A16[:], ps[0:npart, 0:W_], AF.Copy,
                                     bias=0.0, scale=1.0)
                r2 = post.tile([npart, W_], F16, tag=f"r2_{th}")
                nc.vector.tensor_scalar(r2[:], A16[:], 0.5, 1023.75, ALU.mult,
                                        ALU.add)
                nc.vector.tensor_scalar(r2[:], r2[:], -1024.0, 2.0, ALU.add,
                                        ALU.mult)
                r3 = r2.rearrange("p (s l) -> p s l", l=NL)
                a3 = A16.rearrange("p (s l) -> p s l", l=NL)
                mask = post.tile([npart, ns * 8], F16, tag=f"mask_{th}")
                t = 2 * b + th
                nc.vector.scalar_tensor_tensor(
                    mask.rearrange("p (s l) -> p s l", l=8),
                    r3[:, :, 0:8], 0.0, a3[:, :, 1:9], ALU.add, ALU.is_lt,
                    accum_out=sb_stats[0:npart, t : t + 1])
                nc.vector.scalar_tensor_tensor(
                    r2[0:npart, 0 : ns * 8], mask[:], 0.0, qs_pair[th][:],
                    ALU.add, ALU.mult,
                    accum_out=sb_stats[0:npart, 16 + t : 17 + t])

    emit_build(0)
    emit_mms(0)
    emit_build(1)
    emit_mms(1)

    # ---------------- final reduction over partitions ----------------
    ps_fin = psfin.tile([NSTAT, 1], F32, tag="fin")
    nc.tensor.matmul(ps_fin[:], sb_stats[:], sb_onescol[:], start=True,
                     stop=True)
    sb_fin = setup.tile([NSTAT, 1], F32)
    nc.vector.tensor_copy(sb_fin[:], ps_fin[:])
    nc.sync.dma_start(stats_d[:], sb_fin[:])


def _build():
    if "nc" in _CACHE:
        return _CACHE["nc"]
    nc = bacc.Bacc(None, target_bir_lowering=False, debug=False)
    prm_d = nc.dram_tensor("prm", [NV, 4 * B], F32, kind="ExternalInput")
    dmap_d = nc.dram_tensor("dmap", [B, GRID, GRID], F32, kind="ExternalInput")
    stats_d = nc.dram_tensor("stats", [48, 1], F32, kind="ExternalOutput")
    from contextlib import ExitStack

    with tile.TileContext(nc) as tc:
        with ExitStack() as ctx:
            _emit(ctx, tc, prm_d, dmap_d, stats_d)
    if hasattr(nc, "compile"):
        nc.compile()
    else:
        nc.finalize()
    _CACHE["nc"] = nc
    return nc


def _host_combine(stats: np.ndarray) -> np.ndarray:
    """stats: [48] -> 8 dice losses for this core's batches."""
    T = stats[0:16]
    I2 = stats[16:32]
    Qs = stats[32:48]
    dices = []
    for b in range(B):
        Tb = T[2 * b] + T[2 * b + 1]
        I2b = I2[2 * b] + I2[2 * b + 1]
        Ib = 0.5 * (Tb + I2b)
        Qb = 0.5 * (Qs[2 * b] + Qs[2 * b + 1] + 65536.0)
        dices.append((2.0 * Ib + SMOOTH) / (Tb + Qb + SMOOTH))
    return np.array(dices, dtype=np.float32)


def _host_params(pts: np.ndarray) -> np.ndarray:
    """pts [B, NV, 2] -> prm [NV, 4B] f32: (miny, maxy, slope, beta)."""
    pc = np.clip(pts * np.float32(255.0), np.float32(0.0),
                 np.float32(255.0)).astype(np.float32)
    pj = np.roll(pc, 1, axis=1)
    piy, pjy = pc[:, :, 1], pj[:, :, 1]
    pix, pjx = pc[:, :, 0], pj[:, :, 0]
    d = (pjy - piy).astype(np.float32)
    d = (d + (d == 0)).astype(np.float32)
    slope = np.clip((pjx - pix) / d, -1e20, 1e20).astype(np.float32)
    beta = (pix - piy * slope).astype(np.float32)
    miny = np.minimum(piy, pjy)
    maxy = np.maximum(piy, pjy)
    prm = np.stack([miny, maxy, slope, beta], axis=2)  # [B, NV, 4]
    return np.ascontiguousarray(prm.transpose(1, 0, 2).reshape(NV, 4 * B))


def kernel(points: np.ndarray, dmap: np.ndarray) -> np.ndarray:
    pts = np.asarray(points, dtype=np.float32).reshape(64, NV, 2)
    dm = np.asarray(dmap, dtype=np.float32).reshape(64, GRID, GRID)

    in_maps = []
    for r in range(N_CORES):
        sl = slice(r * B, (r + 1) * B)
        in_maps.append({
            "prm": _host_params(pts[sl]),
            "dmap": np.ascontiguousarray(dm[sl]),
        })

    nc = _build()
    res = run_bass_kernel_spmd(nc, in_maps, core_ids=list(range(N_CORES)))

    dices = []
    for r in range(N_CORES):
        s = np.asarray(res.results[r]["stats"], dtype=np.float32).reshape(-1)
        dices.append(_host_combine(s))
    dices = np.concatenate(dices).astype(np.float32)
    return np.float32(np.mean(np.float32(1.0) - dices))
